# revision 9
# baseline (speedup 1.0000x reference)
"""Bundle-adjustment projection kernel for 8 Trainium2 NeuronCores.

out[v, n, :] = (u, v) pixel projection of point n under view v
(reference: nn_BundleAdjustmentModel).

Sharding: data-parallel over views — 8 views per core, points replicated.
Per core the pipeline is pure elementwise work spread across DVE / ACT /
GPSIMD engines (PE matmul loses badly here: K=4 contractions with fp32
need 4 cyc/row plus stationary churn):

  zc = R2.p - depth                  (fp32: ACT init + 2 DVE scalar_tensor_tensor)
  rs = clip(1/zc, +-1/eps)           (DVE reciprocal_approx_fast + GPSIMD clip,
                                      == sign(zc)/max(|zc|, eps))
  a  = (-f*R0.p - f*tx)/256          (fp16 chain, /256 keeps a*rs in fp16 range)
  b  = ( f*R1.p + f*ty)/256          (fp16 chain)
  u  = (a*rs)*256 + cx ; v = (b*rs)*256 + cy   (ACT, interleaved strided write)

Host precomputes the per-view 3x4 affine coefficient rows (folding focal/
softplus/sign), which is O(V) work; all O(V*N) work runs on device.
"""
import sys
import types

import numpy as np

V = 64
N = 500000
NC = 8  # cores
NV_LOC = V // NC  # views per core
TCOLS = 3908  # even (fp16 2x mode) and >= ceil(N/128); 128*3908 = 500224
NPAD = 128 * TCOLS
CHUNK = 1954
AB_SCALE = 256.0
MIN_FOCAL = 50.0
MIN_DISTANCE = 0.25
Z_EPS = 1e-4

_CACHE = {}


def _setup_paths():
    if "/opt/trn_rl_repo" not in sys.path:
        sys.path.insert(0, "/opt/trn_rl_repo")
    # the axon trace path imports antenv.axon_hooks; provide a stub if absent
    try:
        import antenv
        if not hasattr(antenv, "axon_hooks"):
            mod = types.ModuleType("antenv.axon_hooks")
            mod._hook = None
            mod.set_axon_ntff_profile_hook = lambda h: setattr(mod, "_hook", h)
            mod.get_axon_ntff_profile_hook = lambda: mod._hook
            sys.modules["antenv.axon_hooks"] = mod
            antenv.axon_hooks = mod
    except ImportError:
        pass


def _build_nc():
    import concourse.bacc as bacc
    import concourse.mybir as mybir
    from concourse import tile

    dt = mybir.dt
    AF = mybir.ActivationFunctionType
    ALU = mybir.AluOpType

    nc = bacc.Bacc("TRN2", target_bir_lowering=False, debug=False)
    PX = nc.dram_tensor("PX", [128, TCOLS], dt.float32, kind="ExternalInput")
    PY = nc.dram_tensor("PY", [128, TCOLS], dt.float32, kind="ExternalInput")
    PZ = nc.dram_tensor("PZ", [128, TCOLS], dt.float32, kind="ExternalInput")
    MB = nc.dram_tensor("MB", [128, 100], dt.float32, kind="ExternalInput")
    OUT = nc.dram_tensor(
        "OUT", [NV_LOC, 128, 2 * TCOLS], dt.float32, kind="ExternalOutput"
    )

    chunks = [(0, CHUNK), (CHUNK, TCOLS - CHUNK)]

    with tile.TileContext(nc) as tc:
        with (
            tc.tile_pool(name="pts", bufs=1) as ppool,
            tc.tile_pool(name="cst", bufs=1) as cpool,
            tc.tile_pool(name="wrk", bufs=2) as wp,
        ):
            xs = ppool.tile([128, TCOLS], dt.float32)
            ys = ppool.tile([128, TCOLS], dt.float32)
            zs = ppool.tile([128, TCOLS], dt.float32)
            x16 = ppool.tile([128, TCOLS], dt.float16)
            y16 = ppool.tile([128, TCOLS], dt.float16)
            z16 = ppool.tile([128, TCOLS], dt.float16)
            nc.sync.dma_start(out=xs[:], in_=PX.ap())
            nc.sync.dma_start(out=ys[:], in_=PY.ap())
            nc.sync.dma_start(out=zs[:], in_=PZ.ap())
            nc.vector.tensor_copy(x16[:], xs[:])
            nc.vector.tensor_copy(y16[:], ys[:])
            nc.vector.tensor_copy(z16[:], zs[:])
            mb = cpool.tile([128, 100], dt.float32)
            nc.sync.dma_start(out=mb[:], in_=MB.ap())

            def col(j):
                return mb[:, j:j + 1]

            cxv = col(96)
            cyv = col(97)
            zp = col(98)  # 0.0

            for v in range(NV_LOC):
                q = 12 * v
                ma0, ma1, ma2, ma3 = col(q), col(q + 1), col(q + 2), col(q + 3)
                mb0, mb1, mb2, mb3 = col(q + 4), col(q + 5), col(q + 6), col(q + 7)
                mz0, mz1, mz2, mz3 = col(q + 8), col(q + 9), col(q + 10), col(q + 11)
                for (c0, w) in chunks:
                    s = slice(c0, c0 + w)
                    zc = wp.tile([128, CHUNK], dt.float32, name="zc", tag="zc")[:, :w]
                    rs = wp.tile([128, CHUNK], dt.float32, name="rs", tag="rs")[:, :w]
                    r16 = wp.tile([128, CHUNK], dt.float16, name="r16",
                                  tag="r16")[:, :w]
                    ac = wp.tile([128, CHUNK], dt.float16, name="ac", tag="ac")[:, :w]
                    bc = wp.tile([128, CHUNK], dt.float16, name="bc", tag="bc")[:, :w]
                    t2 = wp.tile([128, CHUNK], dt.float16, name="t2", tag="t2")[:, :w]
                    t3 = wp.tile([128, CHUNK], dt.float16, name="t3", tag="t3")[:, :w]
                    t4 = wp.tile([128, CHUNK], dt.float16, name="t4", tag="t4")[:, :w]
                    t5 = wp.tile([128, CHUNK], dt.float16, name="t5", tag="t5")[:, :w]
                    uv = wp.tile([128, 2 * CHUNK], dt.float32, name="uv",
                                 tag="uv")[:, :2 * w]

                    # z chain (fp32): zc = z*Mz2 + Mz3 + x*Mz0 + y*Mz1
                    nc.scalar.activation(zc, zs[:, s], AF.Identity,
                                         scale=mz2, bias=mz3)
                    nc.vector.scalar_tensor_tensor(
                        zc, xs[:, s], mz0, zc, op0=ALU.mult, op1=ALU.add)
                    nc.vector.scalar_tensor_tensor(
                        zc, ys[:, s], mz1, zc, op0=ALU.mult, op1=ALU.add)
                    # safe reciprocal: 1/zc clipped to +-1/eps, cast to fp16
                    nc.vector.reciprocal_approx_fast(out=rs, in_=zc)
                    nc.gpsimd.tensor_scalar(
                        r16, rs, 1.0 / Z_EPS, -1.0 / Z_EPS, ALU.min, ALU.max)
                    # a chain (fp16 /256): ac = (x*ma0+ma3) + y*ma1 + z*ma2
                    nc.scalar.activation(ac, x16[:, s], AF.Identity,
                                         scale=ma0, bias=ma3)
                    nc.vector.tensor_scalar(
                        t2, y16[:, s], ma1, 0.0, ALU.mult, ALU.add)
                    nc.scalar.activation(t3, z16[:, s], AF.Identity,
                                         scale=ma2, bias=0.0)
                    nc.vector.tensor_tensor(ac, ac, t2, ALU.add)
                    nc.vector.tensor_tensor(ac, ac, t3, ALU.add)
                    # b chain (fp16 /256): bc = (y*mb1+mb3) + x*mb0 + z*mb2
                    nc.scalar.activation(bc, y16[:, s], AF.Identity,
                                         scale=mb1, bias=mb3)
                    nc.vector.tensor_scalar(
                        t4, x16[:, s], mb0, 0.0, ALU.mult, ALU.add)
                    nc.scalar.activation(t5, z16[:, s], AF.Identity,
                                         scale=mb2, bias=0.0)
                    nc.vector.tensor_tensor(bc, bc, t4, ALU.add)
                    nc.vector.tensor_tensor(bc, bc, t5, ALU.add)
                    # project (in-place) + interleave with *256 and +cx/+cy
                    nc.vector.tensor_tensor(t2, ac, r16, ALU.mult)
                    nc.gpsimd.tensor_tensor(t4, bc, r16, ALU.mult)
                    uvv = uv.rearrange("p (n two) -> p two n", two=2)
                    nc.scalar.activation(uvv[:, 0, :], t2, AF.Identity,
                                         scale=AB_SCALE, bias=cxv)
                    nc.scalar.activation(uvv[:, 1, :], t4, AF.Identity,
                                         scale=AB_SCALE, bias=cyv)
                    nc.sync.dma_start(
                        out=OUT.ap()[v][:, 2 * c0:2 * (c0 + w)], in_=uv)
    nc.compile()
    return nc


def _host_precompute(points, euler, translation_xy, translation_depth_raw,
                     focal_raw, cx, cy):
    """Replicate the reference's O(V) math in fp32 numpy."""
    euler = np.asarray(euler, np.float32)
    c = np.cos(euler)
    s = np.sin(euler)
    cx_, cy_, cz_ = c[:, 0], c[:, 1], c[:, 2]
    sx_, sy_, sz_ = s[:, 0], s[:, 1], s[:, 2]
    one = np.ones_like(cx_)
    zero = np.zeros_like(cx_)
    rx = np.stack([
        np.stack([one, zero, zero], -1),
        np.stack([zero, cx_, -sx_], -1),
        np.stack([zero, sx_, cx_], -1)], -2).astype(np.float32)
    ry = np.stack([
        np.stack([cy_, zero, sy_], -1),
        np.stack([zero, one, zero], -1),
        np.stack([-sy_, zero, cy_], -1)], -2).astype(np.float32)
    rz = np.stack([
        np.stack([cz_, -sz_, zero], -1),
        np.stack([sz_, cz_, zero], -1),
        np.stack([zero, zero, one], -1)], -2).astype(np.float32)
    rot = np.matmul(np.matmul(rx, ry), rz).astype(np.float32)  # [V,3,3]

    tdr = np.asarray(translation_depth_raw, np.float32)
    depth = (np.logaddexp(tdr, np.float32(0.0)).astype(np.float32)
             + np.float32(MIN_DISTANCE)).astype(np.float32)
    fr = np.float32(np.asarray(focal_raw).reshape(-1)[0])
    focal = np.float32(np.logaddexp(fr, np.float32(0.0))) + np.float32(MIN_FOCAL)
    txy = np.asarray(translation_xy, np.float32)

    # per-view coefficient block: [Ma(4) | Mb(4) | Mz(4)]; a/b rows /256
    M = np.zeros((V, 12), np.float32)
    M[:, 0:3] = (-focal / AB_SCALE) * rot[:, 0, :]
    M[:, 3] = (-focal / AB_SCALE) * txy[:, 0]
    M[:, 4:7] = (focal / AB_SCALE) * rot[:, 1, :]
    M[:, 7] = (focal / AB_SCALE) * txy[:, 1]
    M[:, 8:11] = rot[:, 2, :]
    M[:, 11] = -depth
    return M, np.float32(cx), np.float32(cy)


def kernel(points, euler, translation_xy, translation_depth_raw, focal_raw,
           cx, cy, _trace=False):
    _setup_paths()
    from concourse.bass_utils import run_bass_kernel_spmd

    if "nc" not in _CACHE:
        _CACHE["nc"] = _build_nc()
    nc = _CACHE["nc"]

    points = np.ascontiguousarray(np.asarray(points, np.float32))
    M, cxf, cyf = _host_precompute(
        points, euler, translation_xy, translation_depth_raw, focal_raw, cx, cy)

    pts_pad = np.zeros((NPAD, 3), np.float32)
    pts_pad[:N] = points
    planes = pts_pad.reshape(128, TCOLS, 3)
    px = np.ascontiguousarray(planes[:, :, 0])
    py = np.ascontiguousarray(planes[:, :, 1])
    pz = np.ascontiguousarray(planes[:, :, 2])

    in_maps = []
    for c in range(NC):
        mbrow = np.zeros(100, np.float32)
        mbrow[:96] = M[c * NV_LOC:(c + 1) * NV_LOC].reshape(-1)
        mbrow[96] = cxf
        mbrow[97] = cyf
        mbt = np.ascontiguousarray(
            np.broadcast_to(mbrow, (128, 100)).astype(np.float32))
        in_maps.append({"PX": px, "PY": py, "PZ": pz, "MB": mbt})

    res = run_bass_kernel_spmd(nc, in_maps, list(range(NC)), trace=_trace)
    _CACHE["last_results"] = res

    out = np.empty((V, N, 2), np.float32)
    for c in range(NC):
        o = res.results[c]["OUT"]  # [NV_LOC, 128, 2*TCOLS]
        o = o.reshape(NV_LOC, NPAD, 2)
        out[c * NV_LOC:(c + 1) * NV_LOC] = o[:, :N, :]
    return out
